# revision 42
# baseline (speedup 1.0000x reference)
"""GCNConv (message passing + linear) on 8 Trainium2 NeuronCores.

Strategy (graph/data parallel, per sharding hint):
  - Nodes sorted by (table, in-degree, A-subdegree) and dealt round-robin
    to the 8 cores (core c owns sorted-ranks {s : s % 8 == c}).
  - Source features are gathered DIRECTLY from host-staged bf16 copies of
    x (two tables, int16 gather index limit) with the Q7 dma_gather
    instruction — no device-side table build, so gathers start at t=0.
  - Per-edge symmetric normalization rsqrt(cnt_src*cnt_dst) is computed
    on device from host-staged integer count tensors (bf16-exact) and
    applied to gathered message tiles in bulk on the Vector engine.
  - Segment-sum on the TensorEngine: scaled message tile
    [128 slots, 128 feat] (stationary) x identity (streaming) accumulated
    into PSUM — one slot per owned dst per tile (transpose-accumulate).
  - Final linear via W^T matmul + bias; output is [d_out, local_dst];
    host unpermutes/transposes back to [N, d_out].

The Bass program is rebuilt per distinct edge_index (layout constants are
baked into the instruction stream); all 8 cores share one program and
differ only in their input data.
"""

import numpy as np

try:
    import ml_dtypes

    _BF16 = ml_dtypes.bfloat16
except ImportError:  # pragma: no cover
    _BF16 = None

import concourse.bacc as bacc
import concourse.bass as bass
import concourse.mybir as mybir
import concourse.tile as tile
from concourse.bass_utils import run_bass_kernel_spmd
from concourse.library_config import mlp as _mlp_lib
from concourse.tile_rust import add_dep_helper

P = 128
N_CORES = 8
NZPAD_ROWS = 64  # zero pad rows per table (consecutive band; row-buffer friendly)
TILES_PER_CALL = 8  # gather granularity; 1024 idxs = max per dma_gather call
SPLIT_ROWS_DEFAULT = 32640  # table-A real rows (255 chunks); A size 32768


def _wrap_idx16(linear_idx):
    """[n] int -> [128, n/16] int16 in the 16-partition wrapped, 8x
    replicated layout dma_gather expects (slot i at [i%16, i//16])."""
    n = linear_idx.shape[0]
    assert n % 16 == 0
    w = linear_idx.reshape(-1, 16).T.astype(np.int16)  # [16, n/16]
    return np.tile(w, (8, 1))


def _tiles_of(order, cntA, cntB, N, n_cores):
    """Total (TA, TB) tiles for a candidate staged order."""
    cA = cntA[order]
    cB = cntB[order]
    G = n_cores * P
    TA = TB = 0
    for g in range((N + G - 1) // G):
        sl = slice(G * g, min(G * (g + 1), N))
        TA += max(int(cA[sl].max()), 1)
        TB += int(cB[sl].max())
    return TA, TB


def _pack_order(in_A, cntA, cntB, N, n_cores, SPLIT, atgt=None, deg=None):
    """Node ordering minimizing per-group tile counts.

    Per region (table A dsts occupy staged rows [0, SPLIT), then B), two
    candidate orders are tried: (a) A-major with cntB snake, extracting
    outlier-cntB nodes into B-major groups; (b) when edge rebalancing put
    most dsts exactly on a degree-determined (cntA, cntB) split, sort the
    on-target dsts by degree (both coordinates tight at once) and the
    off-target rest B-major. Best by total tiles wins; finally whole blocks
    of n_cores*P rows are permuted heavy-first WITHIN each region so the
    last processed group has the shortest matmul tail.
    """
    snakeB = np.where(cntA % 2 == 0, cntB, (1 << 20) - cntB)
    snakeA = np.where(cntB % 2 == 0, cntA, (1 << 20) - cntA)

    def region(nodes, tb):
        cB_ = cntB[nodes]
        hiB = cB_ >= tb
        oB = nodes[hiB][np.lexsort((snakeA[nodes[hiB]], cB_[hiB]))]
        rest = nodes[~hiB]
        oR = rest[np.lexsort((snakeB[rest], cntA[rest]))]
        return np.concatenate([oR, oB])

    def region_tgt(nodes):
        tgt = atgt[nodes]
        on = nodes[tgt]
        oT = on[np.lexsort((snakeB[on], deg[on]))]
        off = nodes[~tgt]
        oO = off[np.lexsort((snakeA[off], cntB[off]))]
        return np.concatenate([oT, oO])

    nodesA = np.where(in_A)[0]
    nodesB = np.where(~in_A)[0]
    # the partial final block is pinned last (it holds the pad ranks); give
    # it the lightest B nodes so the closing matmul chain is short
    G = n_cores * P
    tail_n = N - (N // G) * G
    if tail_n:
        light = np.argsort(cntA[nodesB] + cntB[nodesB], kind="stable")[:tail_n]
        tail_nodes = nodesB[light]
        keep = np.ones(nodesB.shape[0], bool)
        keep[light] = False
        nodesB = nodesB[keep]
    else:
        tail_nodes = np.zeros(0, np.int64)
    best = None
    for tb in (6, 7, 8, 9, 10, 12, 1 << 30):
        o = np.concatenate([region(nodesA, tb), region(nodesB, tb), tail_nodes])
        TA, TB = _tiles_of(o, cntA, cntB, N, n_cores)
        if best is None or TA + TB < best[0]:
            best = (TA + TB, o)
    if atgt is not None:
        o = np.concatenate([region_tgt(nodesA), region_tgt(nodesB), tail_nodes])
        TA, TB = _tiles_of(o, cntA, cntB, N, n_cores)
        if TA + TB < best[0]:
            best = (TA + TB, o)
    order = best[1]

    # heavy-first block permutation, respecting the region boundary: blocks
    # entirely inside [0, SPLIT) shuffle among themselves, ditto blocks in
    # [SPLIT, N); the boundary-spanning block and the partial tail block
    # stay in place.
    G = n_cores * P
    ngr = (N + G - 1) // G
    weight = np.zeros(ngr, np.int64)
    cA = cntA[order]
    cB = cntB[order]
    for g in range(ngr):
        sl = slice(G * g, min(G * (g + 1), N))
        weight[g] = max(int(cA[sl].max()), 1) + int(cB[sl].max())
    full = ngr - 1 if N % G else ngr  # partial tail block stays last
    a_end = SPLIT // G  # blocks [0, a_end) are pure-A
    b_start = (SPLIT + G - 1) // G  # blocks [b_start, full) are pure-B
    permA = np.argsort(-weight[:a_end], kind="stable")
    permB = b_start + np.argsort(-weight[b_start:full], kind="stable")
    blocks = [order[G * g : G * (g + 1)] for g in permA]
    if a_end < b_start:
        blocks.append(order[G * a_end : G * b_start])
    blocks += [order[G * g : G * (g + 1)] for g in permB]
    if full < ngr:
        blocks.append(order[G * full :])
    return np.concatenate(blocks)


# ----------------------------------------------------------------------------
# Host-side layout construction (sharding / index relabeling only — all
# floating-point arithmetic happens on device).
# ----------------------------------------------------------------------------
def _prep(x, edge_index, weight, bias, n_cores, split_rows=SPLIT_ROWS_DEFAULT):
    N, D = x.shape
    assert D == P
    src = np.asarray(edge_index[0], dtype=np.int64)
    dst = np.asarray(edge_index[1], dtype=np.int64)
    E = src.shape[0]

    deg = np.bincount(dst, minlength=N)
    count = deg + 1  # self-loop included

    CH = N // P + 1  # staged chunks; >=1 trailing zero row
    NSTAGED = CH * P

    two_tables = split_rows < N
    if two_tables:
        SPLIT = split_rows
        assert SPLIT % P == 0 and SPLIT + P <= 32768
        # table A = the HIGHEST-count nodes: the B region (staged last,
        # processed last) then holds the lightest dsts, giving the final
        # groups — including the forced-last partial block — short matmul
        # tails. Source-side membership stats are degree-independent.
        prelim = np.argsort(count, kind="stable")
        in_A = np.zeros(N, bool)
        in_A[prelim[N - SPLIT :]] = True
        # per-node count of A-source edges (self-loops handled separately)
        cntA = np.bincount(dst[in_A[src]], minlength=N)
        cntB = deg - cntA

        # --- edge rebalancing: duplicate some A sources into table B's
        # spare rows so each dst's (cntA, cntB) can be clamped to a
        # degree-determined split. Sorting by degree then keeps BOTH
        # per-group maxima tight at once, cutting pad tiles.
        NZPAD_ = NZPAD_ROWS
        NB_REAL_ = NSTAGED - SPLIT
        DUP0 = NB_REAL_ + NZPAD_
        DUP_CAP = 32768 - DUP0
        pA = cntA.sum() / max(E, 1)
        sig = np.sqrt(np.maximum(deg * pA * (1 - pA), 0.0))
        cA_t = np.maximum(np.floor(deg * pA - 0.9 * sig), 0).astype(np.int64)
        excess = np.maximum(cntA - cA_t, 0)
        edge_in_B = ~in_A[src]
        duprow = np.full(N, -1, np.int64)
        ndup = 0
        if DUP_CAP > 0 and excess.sum() > 0:
            ea = np.where(~edge_in_B)[0]
            sort_ea = ea[np.argsort(dst[ea], kind="stable")]
            ebounds = np.searchsorted(dst[sort_ea], np.arange(N + 1))
            for dd in np.argsort(-excess, kind="stable"):
                k = int(excess[dd])
                if k == 0:
                    break
                es = sort_ea[ebounds[dd] : ebounds[dd + 1]]
                have = duprow[src[es]] >= 0
                pick = list(es[have][:k])
                need = k - len(pick)
                if need > 0:
                    for e in es[~have][: min(need, DUP_CAP - ndup)]:
                        s_ = src[e]
                        if duprow[s_] < 0:
                            duprow[s_] = DUP0 + ndup
                            ndup += 1
                        pick.append(e)
                edge_in_B[pick] = True
            cntA = np.bincount(dst[~edge_in_B], minlength=N)
            cntB = deg - cntA
        atgt = cntA == cA_t
        order = _pack_order(in_A, cntA, cntB, N, n_cores, SPLIT, atgt, deg)
        NA = SPLIT + P  # pad rows SPLIT..NA-1 are zeros
        NB = NSTAGED - SPLIT
    else:
        SPLIT = NSTAGED  # everything in table A
        in_A = np.ones(N, bool)
        cntA = deg.copy()
        cntB = deg - cntA
        edge_in_B = np.zeros(E, bool)
        duprow = np.full(N, -1, np.int64)
        ndup = 0
        order = np.argsort(count, kind="stable")
        NA = NSTAGED
        NB = 0

    rank = np.empty(N, np.int64)
    rank[order] = np.arange(N)

    count_staged = np.zeros(NSTAGED, np.int64)
    count_staged[:N] = count[order]
    cntA_staged = np.zeros(NSTAGED, np.int64)
    cntA_staged[:N] = cntA[order]
    cntB_staged = np.zeros(NSTAGED, np.int64)
    cntB_staged[:N] = cntB[order]
    x_staged = np.zeros((NSTAGED, D), np.float32)
    x_staged[:N] = np.asarray(x, dtype=np.float32)[order]

    # edges grouped by dst rank, pass-A edges first within each dst
    drank = rank[dst]
    eorder = np.lexsort((edge_in_B, drank))
    es_ = src[eorder]
    eB_ = edge_in_B[eorder]
    # per-edge row within its pass's gather table
    erow = np.where(
        ~eB_,
        rank[es_],
        np.where(~in_A[es_], rank[es_] - SPLIT, duprow[es_]),
    )
    ecnt = count[es_]  # per-edge source count (deg+1)
    deg_by_rank = (count[order] - 1).astype(np.int64)
    starts = np.zeros(N + 1, np.int64)
    starts[1:] = np.cumsum(deg_by_rank)

    LOCAL = (N + n_cores - 1) // n_cores
    GROUPS = (LOCAL + P - 1) // P
    LOCAL_PAD = GROUPS * P

    TgA, TgB = [], []
    for g in range(GROUPS):
        lo = n_cores * P * g
        hi = min(n_cores * P * (g + 1), N)
        if lo < N:
            TgA.append(int(cntA_staged[lo:hi].max()))
            TgB.append(int(cntB_staged[lo:hi].max()))
        else:
            TgA.append(1)
            TgB.append(0)
        if TgA[-1] + TgB[-1] == 0:
            TgA[-1] = 1
    toffsA = np.zeros(GROUPS + 1, np.int64)
    toffsA[1:] = np.cumsum(TgA)
    toffsB = np.zeros(GROUPS + 1, np.int64)
    toffsB[1:] = np.cumsum(TgB)
    T_totalA = int(toffsA[-1])
    T_totalB = int(toffsB[-1])
    T_total = T_totalA + T_totalB

    # pad slots point at guaranteed-zero rows. Spread pads across many
    # distinct zero rows: same-address gather descriptors serialize badly
    # (~5x slower per row than distinct addresses).
    NZPAD = NZPAD_ROWS  # zero rows per table reserved for pads
    if two_tables:
        PAD_A0 = SPLIT  # rows SPLIT..SPLIT+127 are zero
        NB_REAL = NSTAGED - SPLIT
        PAD_B0 = NB_REAL  # then 128 zero pad rows, then duplicated A rows
        NB = NB_REAL + NZPAD + ndup
        assert NB <= 32768
    else:
        PAD_A0 = N
        PAD_B0 = 0
        NB = 0

    x_own = np.zeros((n_cores, GROUPS * P, D), np.float32)
    for c in range(n_cores):
        k = np.arange(min((N - c + n_cores - 1) // n_cores, GROUPS * P))
        s_r = n_cores * k + c
        x_own[c][: k.shape[0]] = x_staged[s_r]

    idxA_cores = np.empty((n_cores, P, 8 * max(T_totalA, 1)), np.int16)
    idxB_cores = np.empty((n_cores, P, 8 * max(T_totalB, 1)), np.int16)
    # per-(slot, tile) integer counts in call-tile layout: A tiles at
    # column toffsA[g]+jj, B tiles at column T_totalA + toffsB[g]+jj
    BIGC = float(2.0**40)  # pad-slot count: nvec = 2^-40 zeroes the message
    cntS_cores = np.full((n_cores, P, T_total), BIGC, np.float32)
    cntD_cores = np.full((n_cores, P, T_total), BIGC, np.float32)
    cntl_cores = np.zeros((n_cores, P, GROUPS), np.float32)  # dst deg (no loop)
    prange = np.arange(P)

    for c in range(n_cores):
        linA = np.full(T_totalA * P, PAD_A0, np.int64)
        linB = np.full(T_totalB * P, PAD_B0, np.int64)
        for g in range(GROUPS):
            s = n_cores * (P * g + prange) + c  # global ranks of this group
            valid = s < N
            sc = np.minimum(s, N - 1)
            ca = np.where(valid, cntA_staged[sc], 0)  # A-source edges
            cb = np.where(valid, cntB_staged[sc], 0)
            st = starts[sc]
            dcnt = np.where(valid, count_staged[sc], 1)  # dst count (deg+1)
            cntl_cores[c][:, g] = np.where(valid, count_staged[sc] - 1, 0)

            # ---- pass A block: tiles toffsA[g] .. +TgA[g]
            TA = TgA[g]
            if TA > 0:
                colsA = np.arange(TA)[None, :]
                pickA = np.minimum(st[:, None] + colsA, max(E - 1, 0))
                takeA = (colsA < ca[:, None]) & valid[:, None]
                valsA = np.where(takeA, erow[pickA], PAD_A0)
                base = int(toffsA[g]) * P
                linA[base : base + TA * P] = valsA.T.ravel()  # tile-major
                c0 = int(toffsA[g])
                cntS_cores[c][:, c0 : c0 + TA] = np.where(
                    takeA, ecnt[pickA], BIGC
                )
                cntD_cores[c][:, c0 : c0 + TA] = np.where(
                    takeA, dcnt[:, None], BIGC
                )

            # ---- pass B block
            TB = TgB[g]
            if TB > 0:
                colsB = np.arange(TB)[None, :]
                pickB = np.minimum(
                    st[:, None] + ca[:, None] + colsB, max(E - 1, 0)
                )
                takeB = (colsB < cb[:, None]) & valid[:, None]
                valsB = np.where(takeB, erow[pickB], PAD_B0)
                base = int(toffsB[g]) * P
                linB[base : base + TB * P] = valsB.T.ravel()
                c0 = T_totalA + int(toffsB[g])
                cntS_cores[c][:, c0 : c0 + TB] = np.where(
                    takeB, ecnt[pickB], BIGC
                )
                cntD_cores[c][:, c0 : c0 + TB] = np.where(
                    takeB, dcnt[:, None], BIGC
                )

        padm = linA == PAD_A0
        linA[padm] = PAD_A0 + (np.arange(int(padm.sum())) % NZPAD)
        assert linA.min() >= 0 and linA.max() < NA
        idxA_cores[c] = _wrap_idx16(linA) if T_totalA else 0
        if T_totalB:
            padm = linB == PAD_B0
            linB[padm] = PAD_B0 + (np.arange(int(padm.sum())) % NZPAD)
            assert linB.min() >= 0 and linB.max() < NB
            idxB_cores[c] = _wrap_idx16(linB)

    # bf16 gather tables (host cast; zero pad rows preserved)
    dup_nodes = np.where(duprow >= 0)[0]
    dup_nodes = dup_nodes[np.argsort(duprow[dup_nodes], kind="stable")]
    xbfA = np.zeros((NA, D), _BF16)
    xbfA[:SPLIT] = x_staged[:SPLIT].astype(_BF16)
    xbfB = np.zeros((max(NB, P), D), _BF16)
    if NB > 0:
        xbfB[: NSTAGED - SPLIT] = x_staged[SPLIT:NSTAGED].astype(_BF16)
        if ndup:
            xbfB[NB - ndup : NB] = np.asarray(x, dtype=np.float32)[
                dup_nodes
            ].astype(_BF16)

    wT = np.ascontiguousarray(np.asarray(weight, dtype=np.float32).T)
    bias_col = np.asarray(bias, dtype=np.float32).reshape(P, 1)

    return dict(
        N=N,
        D=D,
        E=E,
        n_cores=n_cores,
        NSTAGED=NSTAGED,
        SPLIT=SPLIT,
        NA=NA,
        NB=NB,
        ndup=ndup,
        dup_nodes=dup_nodes,
        GROUPS=GROUPS,
        LOCAL=LOCAL,
        LOCAL_PAD=LOCAL_PAD,
        TgA=TgA,
        TgB=TgB,
        toffsA=toffsA,
        toffsB=toffsB,
        T_totalA=T_totalA,
        T_totalB=T_totalB,
        T_total=T_total,
        x_staged=x_staged,
        x_own=x_own,
        xbfA=xbfA,
        xbfB=xbfB,
        cntS_cores=cntS_cores,
        cntD_cores=cntD_cores,
        cntl_cores=cntl_cores,
        idxA_cores=idxA_cores,
        idxB_cores=idxB_cores,
        wT=wT,
        bias_col=bias_col,
        order=order,
    )


# ----------------------------------------------------------------------------
# Device program
# ----------------------------------------------------------------------------
def _build(L):
    NA, NB = L["NA"], L["NB"]
    GROUPS = L["GROUPS"]
    TgA, TgB = L["TgA"], L["TgB"]
    toffsA, toffsB = L["toffsA"], L["toffsB"]
    T_totalA, T_totalB = L["T_totalA"], L["T_totalB"]
    T_total = L["T_total"]
    LOCAL_PAD = L["LOCAL_PAD"]
    f32 = mybir.dt.float32
    bf16 = mybir.dt.bfloat16
    i16 = mybir.dt.int16
    AF = mybir.ActivationFunctionType

    nc = bacc.Bacc("TRN2", debug=False, num_devices=L["n_cores"], num_swdge_queues=4)
    xbfA_dram = nc.dram_tensor("xbfA", [NA, P], bf16, kind="ExternalInput")
    xbfB_dram = nc.dram_tensor("xbfB", [max(NB, P), P], bf16, kind="ExternalInput")
    cntS_dram = nc.dram_tensor("cntS", [P, T_total], bf16, kind="ExternalInput")
    cntD_dram = nc.dram_tensor("cntD", [P, T_total], bf16, kind="ExternalInput")
    cntl_dram = nc.dram_tensor("cntl", [P, GROUPS], bf16, kind="ExternalInput")
    idxA_dram = nc.dram_tensor(
        "idxA", [P, 8 * max(T_totalA, 1)], i16, kind="ExternalInput"
    )
    idxB_dram = nc.dram_tensor(
        "idxB", [P, 8 * max(T_totalB, 1)], i16, kind="ExternalInput"
    )
    xown_dram = nc.dram_tensor("x_own", [LOCAL_PAD, P], f32, kind="ExternalInput")
    wT_dram = nc.dram_tensor("wT", [P, P], f32, kind="ExternalInput")
    identb_dram = nc.dram_tensor("identb", [P, P], bf16, kind="ExternalInput")
    bias_dram = nc.dram_tensor("bias_col", [P, 1], f32, kind="ExternalInput")
    out_dram = nc.dram_tensor("out", [P, LOCAL_PAD], f32, kind="ExternalOutput")

    with tile.TileContext(nc) as tc:
        with (
            tc.tile_pool(name="const", bufs=1) as cpool,
            tc.tile_pool(name="msg", bufs=56) as mpool,
            tc.tile_pool(name="uself", bufs=10) as uspool,
            tc.tile_pool(name="xo", bufs=10) as xopool,
            tc.tile_pool(name="agg", bufs=8) as apool,
            tc.tile_pool(name="outs", bufs=4) as opool,
            tc.tile_pool(name="ps", bufs=6, space="PSUM") as pspool,
            tc.tile_pool(name="ps2", bufs=2, space="PSUM") as ps2pool,
        ):
            lib_inst = nc.gpsimd.load_library(_mlp_lib)

            # ---- idx loads, chunked so early gathers start ASAP
            IDX_CHUNK = 8 * TILES_PER_CALL * 4  # 4 calls per chunk
            idxA_sb = cpool.tile([P, 8 * max(T_totalA, 1)], i16)
            for o in range(0, 8 * max(T_totalA, 1), IDX_CHUNK):
                hi = min(o + IDX_CHUNK, 8 * max(T_totalA, 1))
                nc.sync.dma_start(out=idxA_sb[:, o:hi], in_=idxA_dram[:, o:hi])
            idxB_sb = cpool.tile([P, 8 * max(T_totalB, 1)], i16)
            for o in range(0, 8 * max(T_totalB, 1), IDX_CHUNK):
                hi = min(o + IDX_CHUNK, 8 * max(T_totalB, 1))
                nc.sync.dma_start(out=idxB_sb[:, o:hi], in_=idxB_dram[:, o:hi])
            cntS_sb = cpool.tile([P, T_total], bf16)
            nc.sync.dma_start(out=cntS_sb[:], in_=cntS_dram[:])
            cntD_sb = cpool.tile([P, T_total], bf16)
            nc.sync.dma_start(out=cntD_sb[:], in_=cntD_dram[:])
            cntl_sb = cpool.tile([P, GROUPS], bf16)
            nc.sync.dma_start(out=cntl_sb[:], in_=cntl_dram[:])
            wT_sb = cpool.tile([P, P], f32)
            nc.sync.dma_start(out=wT_sb[:], in_=wT_dram[:])
            bias_sb = cpool.tile([P, 1], f32)
            nc.sync.dma_start(out=bias_sb[:], in_=bias_dram[:])
            identb_sb = cpool.tile([P, P], bf16)
            nc.sync.dma_start(out=identb_sb[:], in_=identb_dram[:])

            # ---- per-edge norm: nvec = rsqrt(cnt_src * cnt_dst)  [P, T_total]
            prod_sb = cpool.tile([P, T_total], f32)
            nc.vector.tensor_tensor(
                out=prod_sb[:],
                in0=cntS_sb[:],
                in1=cntD_sb[:],
                op=mybir.AluOpType.mult,
            )
            nc.scalar.sqrt(prod_sb[:], prod_sb[:])
            nvecf_sb = cpool.tile([P, T_total], f32)
            nc.vector.reciprocal(nvecf_sb[:], prod_sb[:])
            nvec_sb = cpool.tile([P, T_total], bf16)
            nc.vector.tensor_copy(out=nvec_sb[:], in_=nvecf_sb[:])

            # ---- self-loop norm: dinvl2 = 1 / (deg + 1)  [P, GROUPS]
            cntl1_sb = cpool.tile([P, GROUPS], f32)
            nc.vector.tensor_scalar_add(cntl1_sb[:], cntl_sb[:], 1.0)
            dinvl2_sb = cpool.tile([P, GROUPS], f32)
            nc.vector.reciprocal(dinvl2_sb[:], cntl1_sb[:])

            # ---- gather + scale + segment-sum (PE) + linear + bias
            msg_tiles = {}
            qrr = [0]

            def ensure_call(pass_key, k):
                key = (pass_key, k)
                if key in msg_tiles:
                    return
                T_tot = T_totalA if pass_key == "A" else T_totalB
                u_src = xbfA_dram if pass_key == "A" else xbfB_dram
                idx_sb = idxA_sb if pass_key == "A" else idxB_sb
                t0 = k * TILES_PER_CALL
                cnt = min(TILES_PER_CALL, T_tot - t0)
                c0 = (0 if pass_key == "A" else T_totalA) + t0
                m = mpool.tile([P, TILES_PER_CALL, P], bf16)
                g_inst = nc.gpsimd.dma_gather(
                    m[:, :cnt, :],
                    u_src[:, :],
                    idx_sb[:, 8 * t0 : 8 * (t0 + cnt)],
                    cnt * P,
                    cnt * P,
                    P,
                    queue_num=qrr[0] % 4,
                )
                qrr[0] += 1
                add_dep_helper(g_inst.ins, lib_inst.ins, reason="ucode lib before gather")
                nc.vector.tensor_tensor(
                    out=m[:, :cnt, :],
                    in0=m[:, :cnt, :],
                    in1=nvec_sb[:, c0 : c0 + cnt].broadcast_to([P, cnt, P]),
                    op=mybir.AluOpType.mult,
                )
                msg_tiles[key] = m

            # the linear+bias+store stage for group g is emitted DELAY groups
            # late: the in-order PE then never stalls on the DVE psum->agg
            # copy (the copy's latency hides behind the next groups'
            # accumulation matmuls), which otherwise serializes the light
            # closing groups at ~1.5us each.
            DELAY = 2
            ohold = {"t": None, "start": 0}
            aggs = {}

            def emit_tail(g):
                agg = aggs.pop(g)
                psum2 = ps2pool.tile([P, P], f32)
                nc.tensor.matmul(
                    out=psum2[:], lhsT=wT_sb[:], rhs=agg[:], start=True, stop=True
                )
                ob = g % 4
                if ob == 0:
                    out_t = opool.tile([P, 4 * P], f32)
                    ohold["t"] = out_t
                    ohold["start"] = g
                nc.scalar.activation(
                    ohold["t"][:, ob * P : (ob + 1) * P],
                    psum2[:],
                    AF.Identity,
                    bias=bias_sb[:, 0:1],
                )
                if ob == 3 or g == GROUPS - 1:
                    w = (g - ohold["start"] + 1) * P
                    nc.sync.dma_start(
                        out=out_dram[:, ohold["start"] * P : ohold["start"] * P + w],
                        in_=ohold["t"][:, :w],
                    )

            for g in range(GROUPS):
                xo = xopool.tile([P, P], f32)
                nc.sync.dma_start(out=xo[:], in_=xown_dram[g * P : (g + 1) * P, :])
                uself = uspool.tile([P, P], bf16)
                nc.scalar.mul(uself[:], xo[:], dinvl2_sb[:, g : g + 1])
                psum = pspool.tile([P, P], f32)
                j = 0
                for pass_key, Tp, toffs in (
                    ("A", TgA[g], toffsA),
                    ("B", TgB[g], toffsB),
                ):
                    for jj in range(Tp):
                        t = int(toffs[g]) + jj
                        k, kk = divmod(t, TILES_PER_CALL)
                        ensure_call(pass_key, k)
                        nc.tensor.matmul(
                            out=psum[:],
                            lhsT=msg_tiles[(pass_key, k)][:, kk, :],
                            rhs=identb_sb[:],
                            start=(j == 0),
                            stop=False,
                        )
                        j += 1
                nc.tensor.matmul(
                    out=psum[:],
                    lhsT=uself[:],
                    rhs=identb_sb[:],
                    start=(j == 0),
                    stop=True,
                )
                agg = apool.tile([P, P], f32)
                nc.vector.tensor_copy(out=agg[:], in_=psum[:])
                aggs[g] = agg
                if g >= DELAY:
                    emit_tail(g - DELAY)
            for g in range(max(GROUPS - DELAY, 0), GROUPS):
                emit_tail(g)

    nc.compile()
    return nc


def _in_maps(L):
    maps = []
    for c in range(L["n_cores"]):
        maps.append(
            {
                "xbfA": L["xbfA"],
                "xbfB": L["xbfB"],
                "cntS": L["cntS_cores"][c].astype(_BF16),
                "cntD": L["cntD_cores"][c].astype(_BF16),
                "cntl": L["cntl_cores"][c].astype(_BF16),
                "idxA": L["idxA_cores"][c],
                "idxB": L["idxB_cores"][c],
                "x_own": L["x_own"][c],
                "wT": L["wT"],
                "identb": np.eye(P, dtype=_BF16),
                "bias_col": L["bias_col"],
            }
        )
    return maps


def _assemble(L, outs):
    N = L["N"]
    n_cores = L["n_cores"]
    LOCAL = L["LOCAL"]
    order = L["order"]
    res = np.empty((N, P), np.float32)
    for c in range(n_cores):
        oc = np.asarray(outs[c]["out"])  # [128, LOCAL_PAD]
        k = np.arange(LOCAL)
        s = n_cores * k + c
        m = s < N
        res[order[s[m]]] = oc[:, :LOCAL][:, m].T
    return res


_CACHE = {}
LAST_EXEC_NS = None


def kernel(x, edge_index, weight, bias, *, trace=False, n_cores=N_CORES):
    global LAST_EXEC_NS
    x = np.asarray(x, dtype=np.float32)
    edge_index = np.asarray(edge_index)
    weight = np.asarray(weight, dtype=np.float32)
    bias = np.asarray(bias, dtype=np.float32)

    key = hash(edge_index.tobytes()) ^ hash((x.shape, n_cores))
    if key in _CACHE:
        L, nc = _CACHE[key]
        xs = np.zeros((L["NSTAGED"], P), np.float32)
        xs[: L["N"]] = x[L["order"]]
        L["x_staged"] = xs
        L["xbfA"][: L["SPLIT"]] = xs[: L["SPLIT"]].astype(_BF16)
        if L["NB"] > 0:
            L["xbfB"][: L["NSTAGED"] - L["SPLIT"]] = (
                xs[L["SPLIT"] : L["NSTAGED"]].astype(_BF16))
            if L["ndup"]:
                L["xbfB"][L["NB"] - L["ndup"] : L["NB"]] = x[
                    L["dup_nodes"]
                ].astype(_BF16)
        xo = np.zeros((L["n_cores"], L["GROUPS"] * P, P), np.float32)
        for c in range(L["n_cores"]):
            k = np.arange(min((L["N"] - c + L["n_cores"] - 1) // L["n_cores"],
                              L["GROUPS"] * P))
            xo[c][: k.shape[0]] = xs[L["n_cores"] * k + c]
        L["x_own"] = xo
        L["wT"] = np.ascontiguousarray(weight.T)
        L["bias_col"] = bias.reshape(P, 1)
    else:
        L = _prep(x, edge_index, weight, bias, n_cores)
        nc = _build(L)
        _CACHE.clear()
        _CACHE[key] = (L, nc)

    res = run_bass_kernel_spmd(
        nc, _in_maps(L), core_ids=list(range(n_cores)), trace=trace
    )
    LAST_EXEC_NS = res.exec_time_ns
    return _assemble(L, res.results)
